# revision 17
# baseline (speedup 1.0000x reference)
"""Trainium2 Bass kernel for GPT-2 style attention block (B=2, S=2048, D=1024, H=16).

Sharding (8 cores): data-parallel over batch (2) x tensor-parallel over heads (4 per
core). Each core: QKV projection for its 4 heads over the full sequence, full-seq
causal attention (transposed-scores layout: softmax reduction folded into the PV
matmul via a ones-column in V), then a row-parallel partial c_proj over the full
sequence using only this core's 256 rows of c_proj_w. No collectives: the host
sums the 4 per-head-group partials per batch (plus the folded v-bias term), so
each core's span is pure compute with no cross-core sync.

Schedule: work is organized in per-qt rounds (512 query columns each). Inside a
round the score matmuls run one kb-step ahead of the PV matmuls so the in-order
PE queue never stalls on an in-flight exp; projection/c_proj fillers drain at
odd kb steps, weighted toward the exp-paced late rounds. Normalization is fused:
reciprocal of the folded denominator runs in place at partition 64, GpSimd
broadcasts it, and a single tensor_tensor reads the PV PSUM, multiplies by the
reciprocal and writes bf16 attnT (evac+normalize in one op; the final pair uses
a low-latency PE broadcast and unblocks the tail c_proj per 256 columns, with
junk warm matmuls keeping the PE clock at 2.4 GHz through the norm window).

Causal structure: score blocks on the diagonal are shrunk to their unmasked
query range and packed contiguously in PSUM so the exp covers no dead columns.
Masking is post-exp: one broadcast-AP multiply covers both heads' 0/1 triangles.

Compute dtype bf16 (fp32 PSUM accumulation); normalization in fp32; partial
outputs shipped as fp16 to halve DMA, tail halves split across both queues.
"""
import sys
sys.path.insert(0, '/opt/trn_rl_repo')

import numpy as np
import ml_dtypes

import concourse.bass as bass
import concourse.mybir as mybir
import concourse.tile as tile
from concourse import bacc
from concourse.bass_utils import run_bass_kernel_spmd

B, S, D = 2, 2048, 1024
H, HD = 16, 64
NCORES = 8
HPC = H // 4          # heads per core = 4

F32 = mybir.dt.float32
F16 = mybir.dt.float16
BF16 = mybir.dt.bfloat16
ADD = mybir.AluOpType.add
MULT = mybir.AluOpType.mult
EXP = mybir.ActivationFunctionType.Exp
ACOPY = mybir.ActivationFunctionType.Copy


def _emit(nc, tc):
    xT = nc.dram_tensor("xT", [D, S], BF16, kind="ExternalInput").ap()
    w_qk = nc.dram_tensor("w_qk", [D, 512], BF16, kind="ExternalInput").ap()
    w_v = nc.dram_tensor("w_v", [D, 256], BF16, kind="ExternalInput").ap()
    w_p = nc.dram_tensor("w_p", [256, D], BF16, kind="ExternalInput").ap()
    bqk = nc.dram_tensor("bqk", [128, 4], F32, kind="ExternalInput").ap()
    cmask = nc.dram_tensor("cmask", [128, 128], BF16, kind="ExternalInput").ap()
    out = nc.dram_tensor("out", [S, D], F16, kind="ExternalOutput").ap()

    from contextlib import ExitStack
    ctx = ExitStack()
    cst = ctx.enter_context(tc.tile_pool(name="cst", bufs=1))
    pw = ctx.enter_context(tc.tile_pool(name="pw", bufs=2, space="PSUM"))
    pat = ctx.enter_context(tc.tile_pool(name="pat", bufs=2, space="PSUM"))
    psc = ctx.enter_context(tc.tile_pool(name="psc", bufs=2, space="PSUM"))
    sb = ctx.enter_context(tc.tile_pool(name="sb", bufs=4))

    # ---- resident SBUF loads, split across both HWDGE queues (sync + scalar)
    # and ordered so round 0's operands land first ----
    xT_sb = cst.tile([128, 8, S], BF16)
    wqk_sb = cst.tile([128, 8, 512], BF16)
    wv_sb = cst.tile([128, 8, 256], BF16)
    bqk_sb = cst.tile([128, 4], F32)
    cm_sb = cst.tile([128, 128], BF16)
    wp_sb = cst.tile([128, 2, D], BF16)
    xT_r = xT.rearrange("(k p) n -> p k n", p=128)
    wqk_r = w_qk.rearrange("(k p) n -> p k n", p=128)
    wv_r = w_v.rearrange("(k p) n -> p k n", p=128)
    nc.scalar.dma_start(bqk_sb[:], bqk)
    nc.scalar.dma_start(cm_sb[:], cmask)
    # round-0-critical prefix (xT qt0 + wqk) split across both HWDGE queues;
    # the bulk goes as single 3D transfers, one per qt block, fanned across
    # four engine queues so enqueue serialization never gates the stream
    for k in range(0, 8, 2):
        nc.sync.dma_start(xT_sb[:, k, 0:512], xT_r[:, k, 0:512])
        nc.sync.dma_start(wqk_sb[:, k], wqk_r[:, k])
        nc.scalar.dma_start(xT_sb[:, k + 1, 0:512], xT_r[:, k + 1, 0:512])
        nc.scalar.dma_start(wqk_sb[:, k + 1], wqk_r[:, k + 1])
    nc.gpsimd.dma_start(xT_sb[:, :, 512:1024], xT_r[:, :, 512:1024])
    nc.gpsimd.dma_start(xT_sb[:, :, 1024:1536], xT_r[:, :, 1024:1536])
    nc.scalar.dma_start(wv_sb[:], wv_r[:])
    nc.sync.dma_start(xT_sb[:, :, 1536:2048], xT_r[:, :, 1536:2048])
    nc.sync.dma_start(wp_sb[:], w_p.rearrange("(k p) n -> p k n", p=128))

    # PE warmer: dependency-free junk matmuls keep the array busy during the
    # input DMAs so HAM unthrottles before real work arrives
    ones_sb = cst.tile([128, 128], BF16)
    nc.vector.memset(ones_sb[:], 1.0)
    wrow = sb.tile([1, 512], BF16, tag="wrow")
    nc.vector.memset(wrow[:], 1.0)
    warm_ps = pw.tile([128, 512], F32, tag="w", name="warm")
    for _ in range(14):
        nc.tensor.matmul(warm_ps[:], ones_sb[0:1, :], wrow[:],
                         start=True, stop=True)

    # qkT [512, 2048]: rows 0-255 = q^T (4 heads x 64, prescaled 1/8), 256-511 = k^T
    qkT_sb = cst.tile([128, 4, S], BF16)

    def qk_proj(m, qt):
        # q^T (m=0,1) / k^T (m=2,3) for one 512-column sequence block
        ps = pw.tile([128, 512], F32, tag="w", name=f"qk{m}_{qt}")
        for k in range(8):
            nc.tensor.matmul(
                ps[:], wqk_sb[:, k, m * 128:(m + 1) * 128],
                xT_sb[:, k, qt * 512:(qt + 1) * 512],
                start=(k == 0), stop=(k == 7))
        nc.vector.tensor_scalar(
            out=qkT_sb[:, m, qt * 512:(qt + 1) * 512], in0=ps[:],
            scalar1=bqk_sb[:, m:m + 1], scalar2=None, op0=ADD)

    # V with interleaved ones column: V_sb [128, 16, 4*65]
    V_sb = cst.tile([128, 16, HPC * 65], BF16)

    def v_ones():
        nc.vector.memset(
            V_sb[:].rearrange("p m (h c) -> p m h c", c=65)[:, :, :, 64:65], 1.0)

    def v_piece(m):
        ps = pw.tile([128, 512], F32, tag="w", name=f"v{m}")
        for k in range(8):
            nc.tensor.matmul(
                ps[:, :256], xT_sb[:, k, m * 128:(m + 1) * 128], wv_sb[:, k, :],
                start=(k == 0), stop=(k == 7))
        nc.vector.tensor_copy(
            out=V_sb[:, m].rearrange("p (h c) -> p h c", c=65)[:, :, 0:64],
            in_=ps[:, :256].rearrange("p (h c) -> p h c", c=64))

    attnT_sb = cst.tile([128, 2, S], BF16)

    def attend_pair(j, qt, defer_v=(), fillers=None, tail=False):
        # heads 2j (partitions 0-63) and 2j+1 (64-127) interleaved: their K=64
        # score matmuls auto-derive different PE row-groups from base_partition
        # and run concurrently when adjacent in the queue. One key block per
        # step, both heads side by side in one PSUM tile -> one exp call covers
        # the pair and psc stays double-buffered. Head A sits at column 0,
        # head B at column 512: each matmul output must stay inside one PSUM
        # bank, so B's offset is fixed even when the diagonal block is shrunk.
        sub = j
        at = {0: pat.tile([128, 512], F32, tag="at", name=f"atA{j}_{qt}"),
              64: pat.tile([128, 512], F32, tag="at", name=f"atB{j}_{qt}")}
        nkb = 4 * qt + 4

        def scores_kb(kb):
            rel = max(0, kb * 128 - qt * 512)
            wb = 512 - rel
            sc = psc.tile([128, 1024], F32, tag="sc", name=f"sc{j}_{qt}_{kb}")
            for i, po in enumerate((0, 64)):
                nc.tensor.matmul(
                    sc[:, i * 512:i * 512 + wb],
                    qkT_sb[po:po + 64, 2 + sub, kb * 128:(kb + 1) * 128],
                    qkT_sb[po:po + 64, sub, qt * 512 + rel:(qt + 1) * 512],
                    start=True, stop=True)
            pt = sb.tile([128, 1024], BF16, tag="pt", name=f"pt{j}_{qt}_{kb}")
            if rel:
                # strided 3D AP: exp only the two valid wb-wide runs, skipping
                # the dead pad between head A's and head B's shrunk blocks
                nc.scalar.activation(
                    out=pt[:].rearrange("p (b c) -> p b c", c=512)[:, :, 0:wb],
                    in_=sc[:].rearrange("p (b c) -> p b c", c=512)[:, :, 0:wb],
                    func=EXP)
            else:
                nc.scalar.activation(out=pt[:, :1024], in_=sc[:, :1024],
                                     func=EXP)
            if kb * 128 >= qt * 512:  # post-exp triangle zeroing, both heads
                # one broadcast-AP multiply covers both heads' triangles
                pt3 = pt[:].rearrange("p (b c) -> p b c", c=512)[:, :, 0:128]
                cmb = cm_sb[:].unsqueeze(1).broadcast_to([128, 2, 128])
                nc.vector.tensor_tensor(pt3, pt3, cmb, MULT)
            return pt

        def pv_kb(kb, pt):
            rel = max(0, kb * 128 - qt * 512)
            wb = 512 - rel
            for i, po in enumerate((0, 64)):
                h = 2 * j + i
                nc.tensor.matmul(
                    at[po][0:65, rel:512], V_sb[:, kb, h * 65:(h + 1) * 65],
                    pt[:, i * 512:i * 512 + wb],
                    start=(kb == 0), stop=(kb == nkb - 1))

        if defer_v:
            # round 0 only: all scores/exps go first so the first exps aren't
            # queued behind 32 cold-start V matmuls; V lands before the PVs
            pts = [scores_kb(kb) for kb in range(nkb)]
            for m in defer_v:
                v_piece(m)
            for kb in range(nkb):
                pv_kb(kb, pts[kb])
        else:
            # scores run one kb ahead of PV so the PE queue never heads-of-line
            # blocks on the exp in flight; fillers drain at odd kb steps
            nxt = scores_kb(0)
            for kb in range(nkb):
                cur = nxt
                if kb + 1 < nkb:
                    nxt = scores_kb(kb + 1)
                pv_kb(kb, cur)
                if fillers and kb % 2 == 1:
                    fillers.pop(0)()

        # normalization: quick-release the at banks first (two fast DVE casts,
        # so the next pair's PV never stalls on the pat double-buffer), ship
        # the ones-row denominators to partition 0 via tiny sync-queue DMAs
        # (keeps the DVE clear), one reciprocal, then normalize in place.
        den2 = sb.tile([1, 1024], F32, tag="den2")
        nc.vector.tensor_copy(out=den2[:, 0:512], in_=at[0][64:65, :])
        nc.vector.tensor_copy(out=den2[:, 512:1024], in_=at[64][64:65, :])
        for po in (0, 64):
            nc.vector.tensor_copy(
                out=attnT_sb[po:po + 64, sub, qt * 512:(qt + 1) * 512],
                in_=at[po][0:64, :])
        rec2 = sb.tile([1, 1024], F32, tag="rec2")
        nc.vector.reciprocal_approx_fast(rec2[:], den2[:])
        if tail:
            # final pair: junk matmuls keep HAM at 2.4 GHz through the norm
            # window, then a low-latency PE broadcast; the multiplies run per
            # 256-column half so the tail c_proj blocks unblock early. The bc
            # tile comes from the psc pool (that slot's exp reads are done);
            # the warmers get a fresh pw tile so they don't alias a live bank.
            warm2 = pw.tile([128, 512], F32, tag="w", name="warm2")
            for _ in range(10):
                nc.tensor.matmul(warm2[:], ones_sb[0:1, :], wrow[:],
                                 start=True, stop=True)
            rec2b = sb.tile([1, 1024], BF16, tag="rec2b")
            nc.vector.tensor_copy(out=rec2b[:], in_=rec2[:])
            bc = psc.tile([128, 1024], F32, tag="sc", name="bc")
            for po in (0, 64):
                nc.tensor.matmul(bc[:, po * 8:po * 8 + 512], ones_sb[0:1, :],
                                 rec2b[:, po * 8:po * 8 + 512],
                                 start=True, stop=True)
            for half in (0, 1):
                for po in (0, 64):
                    sl = attnT_sb[po:po + 64, sub,
                                  qt * 512 + half * 256:qt * 512 + half * 256 + 256]
                    rb = bc[po:po + 64, po * 8 + half * 256:po * 8 + half * 256 + 256]
                    nc.vector.tensor_tensor(sl, sl, rb, MULT)
                c_proj_tail((12 + 2 * half, 13 + 2 * half), half)
        else:
            # broadcast + in-place normalize entirely on GpSimd (all SBUF):
            # attnT readiness only gates c_proj fillers rounds later
            recb = sb.tile([128, 1024], F32, tag="recb")
            nc.gpsimd.partition_broadcast(recb[:], rec2[:])
            for po in (0, 64):
                sl = attnT_sb[po:po + 64, sub, qt * 512:(qt + 1) * 512]
                rb = recb[po:po + 64, po * 8:po * 8 + 512]
                nc.gpsimd.tensor_tensor(sl, sl, rb, MULT)

    def c_proj(ms):
        # partial c_proj: contract only this core's 256 D-rows (2 u-blocks of
        # 128), full 2048-seq output; host sums partials across head groups.
        for m in ms:
            out_sb = sb.tile([128, D], F16, tag="out")
            ps = [pw.tile([128, 512], F32, tag="w", name=f"pj{m}_{n}") for n in range(2)]
            for u in range(2):
                for n in range(2):
                    nc.tensor.matmul(
                        ps[n][:], attnT_sb[:, u, m * 128:(m + 1) * 128],
                        wp_sb[:, u, n * 512:(n + 1) * 512],
                        start=(u == 0), stop=(u == 1))
            nc.vector.tensor_copy(out=out_sb[:, 0:512], in_=ps[0][:])
            nc.vector.tensor_copy(out=out_sb[:, 512:1024], in_=ps[1][:])
            nc.sync.dma_start(out[m * 128:(m + 1) * 128, :], out_sb[:])

    def c_proj_tail(ms, parity):
        # tail blocks: PSUM evacuation split across Vector and Scalar (both
        # idle by now) and the out halves shipped on alternating queues so the
        # final DMA drain is spread across both HWDGE rings.
        for m in ms:
            out_sb = sb.tile([128, D], F16, tag="out")
            ps = [pw.tile([128, 512], F32, tag="w", name=f"pj{m}_{n}") for n in range(2)]
            for u in range(2):
                for n in range(2):
                    nc.tensor.matmul(
                        ps[n][:], attnT_sb[:, u, m * 128:(m + 1) * 128],
                        wp_sb[:, u, n * 512:(n + 1) * 512],
                        start=(u == 0), stop=(u == 1))
            nc.vector.tensor_copy(out=out_sb[:, 0:512], in_=ps[0][:])
            nc.scalar.activation(out=out_sb[:, 512:1024], in_=ps[1][:],
                                 func=ACOPY)
            qa, qb = (nc.sync, nc.scalar) if m % 2 == 0 else (nc.scalar, nc.sync)
            qa.dma_start(out[m * 128:(m + 1) * 128, 0:512], out_sb[:, 0:512])
            qb.dma_start(out[m * 128:(m + 1) * 128, 512:1024], out_sb[:, 512:1024])

    # ---- per-qt rounds, software-pipelined: the next round's q/k projections
    # and V pieces are emitted before the current round ends so the scalar
    # engine's exp stream never drains at round boundaries; prev-round c_proj
    # blocks fill exp-gated stretches ----
    v_ones()
    qk_proj(0, 0)
    qk_proj(2, 0)
    attend_pair(0, 0, defer_v=(0, 1, 2, 3))
    qk_proj(1, 0)
    qk_proj(3, 0)
    attend_pair(1, 0)
    qk_proj(0, 1)
    qk_proj(2, 1)
    for m in range(4, 8):
        v_piece(m)
    # fillers drain inside the kb loops (at odd kb steps); leftovers are
    # emitted right after, always before any consumer pair. c_proj blocks are
    # weighted toward the exp-paced late rounds where no projection work
    # remains and the PE would otherwise idle.
    filler_plan = {
        (1, 0): [lambda: qk_proj(1, 1), lambda: qk_proj(3, 1)],
        (1, 1): [lambda: qk_proj(0, 2), lambda: qk_proj(2, 2)]
                + [lambda m=m: v_piece(m) for m in range(8, 12)],
        (2, 0): [lambda: qk_proj(1, 2), lambda: qk_proj(3, 2),
                 lambda: c_proj((0,)), lambda: c_proj((1,))],
        (2, 1): [lambda: qk_proj(0, 3), lambda: qk_proj(2, 3),
                 lambda: c_proj((2,)), lambda: c_proj((3,)),
                 lambda: c_proj((4,))],
        (3, 0): [lambda m=m: v_piece(m) for m in range(12, 16)]
                + [lambda: qk_proj(1, 3), lambda: qk_proj(3, 3),
                   lambda: c_proj((5,)), lambda: c_proj((6,))],
        (3, 1): [lambda: c_proj((7,)), lambda: c_proj((8,)),
                 lambda: c_proj((9,)), lambda: c_proj((10,)),
                 lambda: c_proj((11,))],
    }
    for qt in range(1, 4):
        for j in (0, 1):
            f = filler_plan[(qt, j)]
            attend_pair(j, qt, fillers=f, tail=(qt == 3 and j == 1))
            for fn in f:
                fn()

    ctx.close()


def build_nc():
    nc = bacc.Bacc("TRN2", target_bir_lowering=False, debug=False, num_devices=NCORES)
    with tile.TileContext(nc) as tc:
        _emit(nc, tc)
    nc.compile()
    return nc


def shard_inputs(hidden_states, c_attn_w, c_attn_b, c_proj_w, c_proj_b):
    x = np.asarray(hidden_states, np.float32)
    W = np.asarray(c_attn_w, np.float32)
    bqkv = np.asarray(c_attn_b, np.float32)
    Wp = np.asarray(c_proj_w, np.float32)

    wq, wk, wv = W[:, :D] * 0.125, W[:, D:2 * D], W[:, 2 * D:]
    bq, bk = bqkv[:D] * 0.125, bqkv[D:2 * D]

    # 128x128 causal triangle keep-mask: 0 where key (row) > query (col), else 1
    k_i = np.arange(128)[:, None]
    q_i = np.arange(128)[None, :]
    cm = (k_i <= q_i).astype(ml_dtypes.bfloat16)

    in_maps = []
    for c in range(NCORES):
        b, r = divmod(c, 4)
        hs = slice(256 * r, 256 * (r + 1))
        w_qk = np.concatenate([wq[:, hs], wk[:, hs]], axis=1)
        bqk_t = np.concatenate([bq[hs], bk[hs]]).reshape(4, 128).T.copy()
        in_maps.append(dict(
            xT=np.ascontiguousarray(x[b].T).astype(ml_dtypes.bfloat16),
            w_qk=w_qk.astype(ml_dtypes.bfloat16),
            w_v=wv[:, hs].astype(ml_dtypes.bfloat16),
            w_p=np.ascontiguousarray(Wp[hs, :]).astype(ml_dtypes.bfloat16),
            bqk=bqk_t.astype(np.float32),
            cmask=cm,
        ))
    return in_maps


def unshard(results, c_attn_b, c_proj_w, c_proj_b):
    bqkv = np.asarray(c_attn_b, np.float32)
    Wp = np.asarray(c_proj_w, np.float32)
    bp = np.asarray(c_proj_b, np.float32)
    # softmax rows sum to 1, so the v-bias passes through attention unchanged:
    # out = (softmax @ xWv + bv) @ Wp + bp = sum(partials) + bv@Wp + bp
    beff = (bqkv[2 * D:] @ Wp + bp).astype(np.float32)
    full = np.zeros((B, S, D), np.float32)
    for c in range(NCORES):
        b = c // 4
        full[b] += results[c]["out"].astype(np.float32)
    full += beff
    return full


_NC = None


def kernel(**inputs):
    global _NC
    if _NC is None:
        _NC = build_nc()
    in_maps = shard_inputs(**inputs)
    res = run_bass_kernel_spmd(_NC, in_maps, core_ids=list(range(NCORES)))
    return unshard(res.results, inputs["c_attn_b"], inputs["c_proj_w"],
                   inputs["c_proj_b"])


if __name__ == "__main__":
    import jax
    with jax.default_device(jax.devices("cpu")[0]):
        import reference
        inputs = {k: np.asarray(v) for k, v in reference.setup_inputs().items()}
        expected = np.asarray(reference.reference(**inputs))
    actual = kernel(**inputs)
    err = np.abs(actual - expected)
    print("max abs err:", err.max(), "rel:", err.max() / np.abs(expected).max())


# revision 21
# speedup vs baseline: 1.0411x; 1.0411x over previous
"""Trainium2 Bass kernel for GPT-2 style attention block (B=2, S=2048, D=1024, H=16).

Sharding (8 cores): data-parallel over batch (2) x tensor-parallel over heads (4 per
core). Each core: QKV projection for its 4 heads over the full sequence, full-seq
causal attention (transposed-scores layout: softmax reduction folded into the PV
matmul via a ones-column in V), then a row-parallel partial c_proj over the full
sequence using only this core's 256 rows of c_proj_w. No collectives: the host
sums the 4 per-head-group partials per batch (plus the folded v-bias term), so
each core's span is pure compute with no cross-core sync.

Schedule: work is organized in per-qt rounds (512 query columns each). Inside a
round the score matmuls run one kb-step ahead of the PV matmuls so the in-order
PE queue never stalls on an in-flight exp; projection/c_proj fillers drain at
odd kb steps, weighted toward the exp-paced late rounds. Normalization is fused:
reciprocal of the folded denominator runs in place at partition 64, GpSimd
broadcasts it, and a single tensor_tensor reads the PV PSUM, multiplies by the
reciprocal and writes bf16 attnT (evac+normalize in one op; the final pair uses
a low-latency PE broadcast and unblocks the tail c_proj per 256 columns, with
junk warm matmuls keeping the PE clock at 2.4 GHz through the norm window).

Causal structure: score blocks on the diagonal are shrunk to their unmasked
query range and packed contiguously in PSUM so the exp covers no dead columns.
Masking is post-exp: one broadcast-AP multiply covers both heads' 0/1 triangles.

Compute dtype bf16 (fp32 PSUM accumulation); normalization in fp32; partial
outputs shipped as fp16 to halve DMA, tail halves split across both queues.
"""
import sys
sys.path.insert(0, '/opt/trn_rl_repo')

import numpy as np
import ml_dtypes

import concourse.bass as bass
import concourse.mybir as mybir
import concourse.tile as tile
from concourse import bacc
from concourse.bass_utils import run_bass_kernel_spmd

B, S, D = 2, 2048, 1024
H, HD = 16, 64
NCORES = 8
HPC = H // 4          # heads per core = 4

F32 = mybir.dt.float32
F16 = mybir.dt.float16
BF16 = mybir.dt.bfloat16
ADD = mybir.AluOpType.add
MULT = mybir.AluOpType.mult
EXP = mybir.ActivationFunctionType.Exp
ACOPY = mybir.ActivationFunctionType.Copy


def _emit(nc, tc):
    xT = nc.dram_tensor("xT", [D, S], BF16, kind="ExternalInput").ap()
    w_qk = nc.dram_tensor("w_qk", [D, 512], BF16, kind="ExternalInput").ap()
    w_v = nc.dram_tensor("w_v", [D, 256], BF16, kind="ExternalInput").ap()
    w_p = nc.dram_tensor("w_p", [256, D], BF16, kind="ExternalInput").ap()
    bqk = nc.dram_tensor("bqk", [128, 4], F32, kind="ExternalInput").ap()
    cmask = nc.dram_tensor("cmask", [128, 128], BF16, kind="ExternalInput").ap()
    out = nc.dram_tensor("out", [S, D], F16, kind="ExternalOutput").ap()

    from contextlib import ExitStack
    ctx = ExitStack()
    cst = ctx.enter_context(tc.tile_pool(name="cst", bufs=1))
    pw = ctx.enter_context(tc.tile_pool(name="pw", bufs=2, space="PSUM"))
    pat = ctx.enter_context(tc.tile_pool(name="pat", bufs=2, space="PSUM"))
    psc = ctx.enter_context(tc.tile_pool(name="psc", bufs=2, space="PSUM"))
    sb = ctx.enter_context(tc.tile_pool(name="sb", bufs=4))

    # ---- resident SBUF loads, split across both HWDGE queues (sync + scalar)
    # and ordered so round 0's operands land first ----
    xT_sb = cst.tile([128, 8, S], BF16)
    wqk_sb = cst.tile([128, 8, 512], BF16)
    wv_sb = cst.tile([128, 8, 256], BF16)
    bqk_sb = cst.tile([128, 4], F32)
    cm_sb = cst.tile([128, 128], BF16)
    wp_sb = cst.tile([128, 2, D], BF16)
    xT_r = xT.rearrange("(k p) n -> p k n", p=128)
    wqk_r = w_qk.rearrange("(k p) n -> p k n", p=128)
    wv_r = w_v.rearrange("(k p) n -> p k n", p=128)
    nc.scalar.dma_start(bqk_sb[:], bqk)
    nc.scalar.dma_start(cm_sb[:], cmask)
    # round-0-critical prefix (xT qt0 + wqk) split across both HWDGE queues;
    # the bulk goes as single 3D transfers, one per qt block, fanned across
    # four engine queues so enqueue serialization never gates the stream
    for k in range(0, 8, 2):
        nc.sync.dma_start(xT_sb[:, k, 0:512], xT_r[:, k, 0:512])
        nc.sync.dma_start(wqk_sb[:, k], wqk_r[:, k])
        nc.scalar.dma_start(xT_sb[:, k + 1, 0:512], xT_r[:, k + 1, 0:512])
        nc.scalar.dma_start(wqk_sb[:, k + 1], wqk_r[:, k + 1])
    nc.sync.dma_start(xT_sb[:, :, 512:1024], xT_r[:, :, 512:1024])
    nc.scalar.dma_start(wv_sb[:], wv_r[:])
    nc.scalar.dma_start(xT_sb[:, :, 1024:1536], xT_r[:, :, 1024:1536])
    # late-needed loads ride the slow SWDGE (gpsimd) queue
    nc.gpsimd.dma_start(xT_sb[:, :, 1536:2048], xT_r[:, :, 1536:2048])
    nc.gpsimd.dma_start(wp_sb[:], w_p.rearrange("(k p) n -> p k n", p=128))

    # PE warmer: dependency-free junk matmuls keep the array busy during the
    # input DMAs so HAM unthrottles before real work arrives
    ones_sb = cst.tile([128, 128], BF16)
    nc.vector.memset(ones_sb[:], 1.0)
    wrow = sb.tile([1, 512], BF16, tag="wrow")
    nc.vector.memset(wrow[:], 1.0)
    warm_ps = pw.tile([128, 512], F32, tag="w", name="warm")
    for _ in range(14):
        nc.tensor.matmul(warm_ps[:], ones_sb[0:1, :], wrow[:],
                         start=True, stop=True)

    # qkT [512, 2048]: rows 0-255 = q^T (4 heads x 64, prescaled 1/8), 256-511 = k^T
    qkT_sb = cst.tile([128, 4, S], BF16)

    def qk_proj(m, qt):
        # q^T (m=0,1) / k^T (m=2,3) for one 512-column sequence block
        ps = pw.tile([128, 512], F32, tag="w", name=f"qk{m}_{qt}")
        for k in range(8):
            nc.tensor.matmul(
                ps[:], wqk_sb[:, k, m * 128:(m + 1) * 128],
                xT_sb[:, k, qt * 512:(qt + 1) * 512],
                start=(k == 0), stop=(k == 7))
        nc.vector.tensor_scalar(
            out=qkT_sb[:, m, qt * 512:(qt + 1) * 512], in0=ps[:],
            scalar1=bqk_sb[:, m:m + 1], scalar2=None, op0=ADD)

    # V with interleaved ones column: V_sb [128, 16, 4*65]
    V_sb = cst.tile([128, 16, HPC * 65], BF16)

    def v_ones():
        nc.vector.memset(
            V_sb[:].rearrange("p m (h c) -> p m h c", c=65)[:, :, :, 64:65], 1.0)

    def v_piece(m):
        ps = pw.tile([128, 512], F32, tag="w", name=f"v{m}")
        for k in range(8):
            nc.tensor.matmul(
                ps[:, :256], xT_sb[:, k, m * 128:(m + 1) * 128], wv_sb[:, k, :],
                start=(k == 0), stop=(k == 7))
        nc.vector.tensor_copy(
            out=V_sb[:, m].rearrange("p (h c) -> p h c", c=65)[:, :, 0:64],
            in_=ps[:, :256].rearrange("p (h c) -> p h c", c=64))

    attnT_sb = cst.tile([128, 2, S], BF16)

    def attend_pair(j, qt, defer_v=(), fillers=None, tail=False):
        # heads 2j (partitions 0-63) and 2j+1 (64-127) interleaved: their K=64
        # score matmuls auto-derive different PE row-groups from base_partition
        # and run concurrently when adjacent in the queue. One key block per
        # step, both heads side by side in one PSUM tile -> one exp call covers
        # the pair and psc stays double-buffered. Head A sits at column 0,
        # head B at column 512: each matmul output must stay inside one PSUM
        # bank, so B's offset is fixed even when the diagonal block is shrunk.
        sub = j
        at = {0: pat.tile([128, 512], F32, tag="at", name=f"atA{j}_{qt}"),
              64: pat.tile([128, 512], F32, tag="at", name=f"atB{j}_{qt}")}
        nkb = 4 * qt + 4

        def scores_kb(kb):
            rel = max(0, kb * 128 - qt * 512)
            wb = 512 - rel
            sc = psc.tile([128, 1024], F32, tag="sc", name=f"sc{j}_{qt}_{kb}")
            for i, po in enumerate((0, 64)):
                nc.tensor.matmul(
                    sc[:, i * 512:i * 512 + wb],
                    qkT_sb[po:po + 64, 2 + sub, kb * 128:(kb + 1) * 128],
                    qkT_sb[po:po + 64, sub, qt * 512 + rel:(qt + 1) * 512],
                    start=True, stop=True)
            pt = sb.tile([128, 1024], BF16, tag="pt", name=f"pt{j}_{qt}_{kb}")
            if rel:
                # strided 3D AP: exp only the two valid wb-wide runs, skipping
                # the dead pad between head A's and head B's shrunk blocks
                nc.scalar.activation(
                    out=pt[:].rearrange("p (b c) -> p b c", c=512)[:, :, 0:wb],
                    in_=sc[:].rearrange("p (b c) -> p b c", c=512)[:, :, 0:wb],
                    func=EXP)
            else:
                nc.scalar.activation(out=pt[:, :1024], in_=sc[:, :1024],
                                     func=EXP)
            if kb * 128 >= qt * 512:  # post-exp triangle zeroing, both heads
                # one broadcast-AP multiply covers both heads' triangles
                pt3 = pt[:].rearrange("p (b c) -> p b c", c=512)[:, :, 0:128]
                cmb = cm_sb[:].unsqueeze(1).broadcast_to([128, 2, 128])
                nc.vector.tensor_tensor(pt3, pt3, cmb, MULT)
            return pt

        def pv_kb(kb, pt):
            rel = max(0, kb * 128 - qt * 512)
            wb = 512 - rel
            for i, po in enumerate((0, 64)):
                h = 2 * j + i
                nc.tensor.matmul(
                    at[po][0:65, rel:512], V_sb[:, kb, h * 65:(h + 1) * 65],
                    pt[:, i * 512:i * 512 + wb],
                    start=(kb == 0), stop=(kb == nkb - 1))

        if defer_v:
            # round 0 only: all scores/exps go first so the first exps aren't
            # queued behind 32 cold-start V matmuls; V lands before the PVs
            pts = [scores_kb(kb) for kb in range(nkb)]
            for m in defer_v:
                v_piece(m)
            for kb in range(nkb):
                pv_kb(kb, pts[kb])
        else:
            # scores run one kb ahead of PV so the PE queue never heads-of-line
            # blocks on the exp in flight; fillers drain at odd kb steps
            # fillers stay clear of the last kb steps so the vector queue is
            # empty when the at-bank evacuations arrive (the pat double-buffer
            # aliases consecutive pairs, so release latency gates the next
            # pair's first PV); leftovers run at the boundary instead
            nxt = scores_kb(0)
            for kb in range(nkb):
                cur = nxt
                if kb + 1 < nkb:
                    nxt = scores_kb(kb + 1)
                pv_kb(kb, cur)
                if fillers and kb % 2 == 1 and kb <= nkb - 4:
                    fillers.pop(0)()

        # normalization: quick-release the at banks first (fast casts at the
        # head of the queue, so the next pair's PV never stalls on the pat
        # double-buffer; the tail splits them across scalar+vector), then the
        # denominator copies, one reciprocal, then normalize in place.
        for po in (0, 64):
            sl = attnT_sb[po:po + 64, sub, qt * 512:(qt + 1) * 512]
            if tail and po == 0:
                nc.scalar.activation(out=sl, in_=at[po][0:64, :], func=ACOPY)
            else:
                nc.vector.tensor_copy(out=sl, in_=at[po][0:64, :])
        den2 = sb.tile([1, 1024], F32, tag="den2")
        nc.vector.tensor_copy(out=den2[:, 0:512], in_=at[0][64:65, :])
        nc.vector.tensor_copy(out=den2[:, 512:1024], in_=at[64][64:65, :])
        rec2 = sb.tile([1, 1024], F32, tag="rec2")
        nc.vector.reciprocal_approx_fast(rec2[:], den2[:])
        if tail:
            # final pair: junk matmuls keep HAM at 2.4 GHz through the norm
            # window, then a low-latency PE broadcast; the multiplies run per
            # 256-column half so the tail c_proj blocks unblock early. The bc
            # tile comes from the psc pool (that slot's exp reads are done);
            # the warmers get a fresh pw tile so they don't alias a live bank.
            warm2 = pw.tile([128, 512], F32, tag="w", name="warm2")
            for _ in range(16):
                nc.tensor.matmul(warm2[:], ones_sb[0:1, :], wrow[:],
                                 start=True, stop=True)
            rec2b = sb.tile([1, 1024], BF16, tag="rec2b")
            nc.vector.tensor_copy(out=rec2b[:], in_=rec2[:])
            bc = psc.tile([128, 1024], F32, tag="sc", name="bc")
            for po in (0, 64):
                nc.tensor.matmul(bc[:, po * 8:po * 8 + 512], ones_sb[0:1, :],
                                 rec2b[:, po * 8:po * 8 + 512],
                                 start=True, stop=True)
            for half in (0, 1):
                for po in (0, 64):
                    sl = attnT_sb[po:po + 64, sub,
                                  qt * 512 + half * 256:qt * 512 + half * 256 + 256]
                    rb = bc[po:po + 64, po * 8 + half * 256:po * 8 + half * 256 + 256]
                    nc.vector.tensor_tensor(sl, sl, rb, MULT)
                c_proj_tail((12 + 2 * half, 13 + 2 * half), half)
        else:
            # broadcast + in-place normalize entirely on GpSimd (all SBUF):
            # attnT readiness only gates c_proj fillers rounds later
            recb = sb.tile([128, 1024], F32, tag="recb")
            nc.gpsimd.partition_broadcast(recb[:], rec2[:])
            for po in (0, 64):
                sl = attnT_sb[po:po + 64, sub, qt * 512:(qt + 1) * 512]
                rb = recb[po:po + 64, po * 8:po * 8 + 512]
                nc.gpsimd.tensor_tensor(sl, sl, rb, MULT)

    def c_proj(ms):
        # partial c_proj: contract only this core's 256 D-rows (2 u-blocks of
        # 128), full 2048-seq output; host sums partials across head groups.
        for m in ms:
            out_sb = sb.tile([128, D], F16, tag="out")
            ps = [pw.tile([128, 512], F32, tag="w", name=f"pj{m}_{n}") for n in range(2)]
            for u in range(2):
                for n in range(2):
                    nc.tensor.matmul(
                        ps[n][:], attnT_sb[:, u, m * 128:(m + 1) * 128],
                        wp_sb[:, u, n * 512:(n + 1) * 512],
                        start=(u == 0), stop=(u == 1))
            nc.vector.tensor_copy(out=out_sb[:, 0:512], in_=ps[0][:])
            nc.vector.tensor_copy(out=out_sb[:, 512:1024], in_=ps[1][:])
            nc.sync.dma_start(out[m * 128:(m + 1) * 128, :], out_sb[:])

    def c_proj_tail(ms, parity):
        # tail blocks: PSUM evacuation split across Vector and Scalar (both
        # idle by now) and the out halves shipped on alternating queues so the
        # final DMA drain is spread across both HWDGE rings.
        for m in ms:
            out_sb = sb.tile([128, D], F16, tag="out")
            ps = [pw.tile([128, 512], F32, tag="w", name=f"pj{m}_{n}") for n in range(2)]
            for u in range(2):
                for n in range(2):
                    nc.tensor.matmul(
                        ps[n][:], attnT_sb[:, u, m * 128:(m + 1) * 128],
                        wp_sb[:, u, n * 512:(n + 1) * 512],
                        start=(u == 0), stop=(u == 1))
            nc.vector.tensor_copy(out=out_sb[:, 0:512], in_=ps[0][:])
            nc.scalar.activation(out=out_sb[:, 512:1024], in_=ps[1][:],
                                 func=ACOPY)
            qa, qb = (nc.sync, nc.scalar) if m % 2 == 0 else (nc.scalar, nc.sync)
            qa.dma_start(out[m * 128:(m + 1) * 128, 0:512], out_sb[:, 0:512])
            qb.dma_start(out[m * 128:(m + 1) * 128, 512:1024], out_sb[:, 512:1024])

    # ---- per-qt rounds, software-pipelined: the next round's q/k projections
    # and V pieces are emitted before the current round ends so the scalar
    # engine's exp stream never drains at round boundaries; prev-round c_proj
    # blocks fill exp-gated stretches ----
    v_ones()
    qk_proj(0, 0)
    qk_proj(2, 0)
    attend_pair(0, 0, defer_v=(0, 1, 2, 3))
    qk_proj(1, 0)
    qk_proj(3, 0)
    attend_pair(1, 0)
    qk_proj(0, 1)
    qk_proj(2, 1)
    for m in range(4, 8):
        v_piece(m)
    # fillers drain inside the kb loops (at odd kb steps); leftovers are
    # emitted right after, always before any consumer pair. c_proj blocks are
    # weighted toward the exp-paced late rounds where no projection work
    # remains and the PE would otherwise idle.
    filler_plan = {
        (1, 0): [lambda: qk_proj(1, 1), lambda: qk_proj(3, 1)],
        (1, 1): [lambda: qk_proj(0, 2), lambda: qk_proj(2, 2)]
                + [lambda m=m: v_piece(m) for m in range(8, 12)],
        (2, 0): [lambda: qk_proj(1, 2), lambda: qk_proj(3, 2),
                 lambda: c_proj((0,)), lambda: c_proj((1,))],
        (2, 1): [lambda: qk_proj(0, 3), lambda: qk_proj(2, 3),
                 lambda: c_proj((2,)), lambda: c_proj((3,)),
                 lambda: c_proj((4,))],
        (3, 0): [lambda m=m: v_piece(m) for m in range(12, 16)]
                + [lambda: qk_proj(1, 3), lambda: qk_proj(3, 3),
                   lambda: c_proj((5,)), lambda: c_proj((6,))],
        (3, 1): [lambda: c_proj((7,)), lambda: c_proj((8,)),
                 lambda: c_proj((9,)), lambda: c_proj((10,)),
                 lambda: c_proj((11,))],
    }
    for qt in range(1, 4):
        for j in (0, 1):
            f = filler_plan[(qt, j)]
            attend_pair(j, qt, fillers=f, tail=(qt == 3 and j == 1))
            for fn in f:
                fn()

    ctx.close()


def build_nc():
    nc = bacc.Bacc("TRN2", target_bir_lowering=False, debug=False, num_devices=NCORES)
    with tile.TileContext(nc) as tc:
        _emit(nc, tc)
    nc.compile()
    return nc


def shard_inputs(hidden_states, c_attn_w, c_attn_b, c_proj_w, c_proj_b):
    x = np.asarray(hidden_states, np.float32)
    W = np.asarray(c_attn_w, np.float32)
    bqkv = np.asarray(c_attn_b, np.float32)
    Wp = np.asarray(c_proj_w, np.float32)

    wq, wk, wv = W[:, :D] * 0.125, W[:, D:2 * D], W[:, 2 * D:]
    bq, bk = bqkv[:D] * 0.125, bqkv[D:2 * D]

    # 128x128 causal triangle keep-mask: 0 where key (row) > query (col), else 1
    k_i = np.arange(128)[:, None]
    q_i = np.arange(128)[None, :]
    cm = (k_i <= q_i).astype(ml_dtypes.bfloat16)

    in_maps = []
    for c in range(NCORES):
        b, r = divmod(c, 4)
        hs = slice(256 * r, 256 * (r + 1))
        w_qk = np.concatenate([wq[:, hs], wk[:, hs]], axis=1)
        bqk_t = np.concatenate([bq[hs], bk[hs]]).reshape(4, 128).T.copy()
        in_maps.append(dict(
            xT=np.ascontiguousarray(x[b].T).astype(ml_dtypes.bfloat16),
            w_qk=w_qk.astype(ml_dtypes.bfloat16),
            w_v=wv[:, hs].astype(ml_dtypes.bfloat16),
            w_p=np.ascontiguousarray(Wp[hs, :]).astype(ml_dtypes.bfloat16),
            bqk=bqk_t.astype(np.float32),
            cmask=cm,
        ))
    return in_maps


def unshard(results, c_attn_b, c_proj_w, c_proj_b):
    bqkv = np.asarray(c_attn_b, np.float32)
    Wp = np.asarray(c_proj_w, np.float32)
    bp = np.asarray(c_proj_b, np.float32)
    # softmax rows sum to 1, so the v-bias passes through attention unchanged:
    # out = (softmax @ xWv + bv) @ Wp + bp = sum(partials) + bv@Wp + bp
    beff = (bqkv[2 * D:] @ Wp + bp).astype(np.float32)
    full = np.zeros((B, S, D), np.float32)
    for c in range(NCORES):
        b = c // 4
        full[b] += results[c]["out"].astype(np.float32)
    full += beff
    return full


_NC = None


def kernel(**inputs):
    global _NC
    if _NC is None:
        _NC = build_nc()
    in_maps = shard_inputs(**inputs)
    res = run_bass_kernel_spmd(_NC, in_maps, core_ids=list(range(NCORES)))
    return unshard(res.results, inputs["c_attn_b"], inputs["c_proj_w"],
                   inputs["c_proj_b"])


if __name__ == "__main__":
    import jax
    with jax.default_device(jax.devices("cpu")[0]):
        import reference
        inputs = {k: np.asarray(v) for k, v in reference.setup_inputs().items()}
        expected = np.asarray(reference.reference(**inputs))
    actual = kernel(**inputs)
    err = np.abs(actual - expected)
    print("max abs err:", err.max(), "rel:", err.max() / np.abs(expected).max())
